# revision 42
# baseline (speedup 1.0000x reference)
"""Multi-head causal attention on 8 Trainium2 NeuronCores.

Problem: B=4, S=2048, d_model=512, H=8 heads, d_k=64, fp32, causal,
scale = 1/sqrt(d_model) (faithful source quirk).

Sharding: 32 (batch, head-group) units -> core c handles batch c%4 and
head group c//4 (4 heads = 256 projection columns). Each core computes
q/k/v projections for its column slice, causal attention for its 4
heads, and a partial output projection (its 256 rows of Wo). The host
sums the two partials per batch and adds the output bias (with bv@Wo
pre-folded into it -- softmax weights sum to 1, so the V bias passes
through attention unchanged).

Phase A streams inputs on three DMA rings at once (sync: KT + low VT,
scalar: QT, gpsimd: pre-transposed weights + high VT) in half-slab
granularity, and emits projection groups t-major so the PE starts as
soon as the first half-slabs land. K copybacks run on ACT (Identity
with per-partition bias), Q copybacks on the DVE, in parallel.

Scores use per-head q/k duplicated onto both partition halves (the 2x
is folded into SCALE/8 on the host): matmul cost is per streamed
column either way, but 64-row matmuls read as a half-idle array to the
HAM clock governor, which then duty-cycles the PE at 1.2 GHz (measured
16 windows full clock / 23 windows half clock, +30us). The 128-row form
keeps the PE at 2.4 GHz.

Phase B is software-pipelined: the scores matmul for chunk i+1 is
emitted on the PE BEFORE the PV matmul of chunk i, so the PE computes
the next block's scores while the exp of the current one runs. The exp
is split across TWO engines: the ACT engine (hardware exp(4v + ln g))
and the DVE, which evaluates exp(4v)*g as (((v+a)v+b)v+c)^4 -- a monic
cubic minimax fit of g^(1/4)*e^v on |v|<=0.78 -- in a single custom-DVE
instruction. The monic normalization constant g cancels in the softmax.

Per head the chunks run jt=0 first (q in [q0,1024), kb 0-7) then jt=1
(q in [1024,2048), kb 0-15), accumulating into four [65,512] PSUM
quarters. Normalize = reciprocal of the sums row -> partition broadcast
-> multiply straight from PSUM. For the last head the high-q quarters
normalize in 128-column pieces as each key block retires, with the
output projection tile for that q-block following immediately.
"""

import sys

sys.path.insert(0, "/opt/trn_rl_repo")

from contextlib import ExitStack

import numpy as np

import concourse.bass as bass
import concourse.tile as tile
from concourse import bacc, mybir
from concourse.bass_utils import run_bass_kernel_spmd

FP32 = mybir.dt.float32
FP16 = mybir.dt.float16
MM = FP16  # matmul operand dtype
MM_NP = np.float16
AF = mybir.ActivationFunctionType

B, S, DM, H = 4, 2048, 512, 8
DK = DM // H  # 64
HC = 4  # heads per core
COLS = HC * DK  # 256
P = 128
NKB = S // P  # 16 key blocks
SCALE = 1.0 / float(np.sqrt(np.float32(DM)))
LAST_KB = (3, 7, 11, 15)  # last kb contributing to each q-quarter
# jt=1 chunks whose exp runs on the DVE, alternating with ACT chunks so
# neither exp engine starves the scores PSUM ring.
# OFF-DIAGONAL ONLY: the poly overflows on -30000 masked diag entries.
KB_DVE = (0, 2, 4, 6)
# diag chunks whose off-diagonal tail (cols 128:w) runs on the DVE; their
# PV is split at the same boundary so each PV piece gates on its own exp
# engine. Relieves ACT in the all-diag stretches (jt0 and kb8-15).
SPLIT_DVE = {(0, 1), (0, 2), (0, 3), (1, 9), (1, 11), (1, 13)}
# narrow chunks sharing one scores tile and ONE merged exp call with their
# predecessor -- halves the ACT per-call fixed cost (~293ns) where the
# chunks are too small to amortize it
PAIR_SECOND = {(0, 5): (0, 4), (0, 7): (0, 6)}

# monic cubic minimax fit of g^(1/4) * e^v on [-0.78, 0.78]:
# m(v) = ((v + PA)*v + PB)*v + PC,  m(v)^4 = g * e^(4v) (rel err < 8e-3)
PA = 3.243170435898654
PB = 6.2111458766350705
PC = 6.176377242985076
G = 1464.1623445051969
LN_G = float(np.log(G))

_CACHED_NC = None
_EXP4_OP = None
_DIVF_OP = None

# 1-NR approximate-divide constants (the Chebyshev pair from
# RECIP_APPROX_FAST is already 1-pass-optimal: max rel err ~0.17%)
DIV_C0 = -0.23549792
DIV_C1 = 2.0017324


def _register_divf():
    """Register DIV_APPROX_ANT: out = Src0 * recip_approx(Src1), where the
    reciprocal is a BITWISE_NOT seed + one inline Newton pass (6 slices).
    Replaces the separate reciprocal+multiply pair in softmax normalize."""
    global _DIVF_OP
    if _DIVF_OP is not None:
        return _DIVF_OP
    import concourse.dve_ops as dve_ops
    from concourse.dve_ops import DveOp
    from concourse.dve_spec import Spec, Src0, Src1, C0, C1, AluOp, Bin, lower
    from concourse.dve_uop import DveOpSpec

    for op in dve_ops.OPS:
        if op.name == "DIV_APPROX_ANT":
            _DIVF_OP = op
            return op

    def _ref_divf(in0, in1, s0, s1, imm2):
        not_d = (~in1.view(np.int32)).view(np.float32)
        y0 = not_d * np.float32(s0)
        y1 = y0 * (np.float32(s1) - in1 * y0)
        return in0 * y1

    _nd = Bin(AluOp.BITWISE_NOT, Src1, Src1)
    _z0 = _nd * C0
    _z1 = _z0 * (C1 - Src1 * _z0)
    spec = Spec(body=Src0 * _z1, reference=_ref_divf)
    shas = {}
    for ver in ("v3", "v4"):
        try:
            tmp = DveOpSpec(
                name="DIV_APPROX_ANT",
                opcode=len(dve_ops.OPS) + 1,
                uops=lower(spec, ver=ver),
                rd1_en=True,
            )
            shas[ver] = tmp.sha(ver)
        except Exception:
            pass
    op = DveOp("DIV_APPROX_ANT", spec, subdim=False, uops_sha=shas)
    dve_ops.OPS.append(op)
    dve_ops._SUB_OPCODE_FOR_NAME[op.name] = len(dve_ops.OPS)
    dve_ops.CUSTOM_DVE_SPECS[op.name] = spec
    _DIVF_OP = op
    return op


def _register_exp4():
    """Register the custom-DVE op EXP4_POLY_ANT (idempotent)."""
    global _EXP4_OP
    if _EXP4_OP is not None:
        return _EXP4_OP
    import concourse.dve_ops as dve_ops
    from concourse.dve_ops import DveOp
    from concourse.dve_spec import Spec, Src0, C0, C1, C2, lower, sq
    from concourse.dve_uop import DveOpSpec

    for op in dve_ops.OPS:
        if op.name == "EXP4_POLY_ANT":
            _EXP4_OP = op
            return op

    def _ref_exp4(in0, in1, s0, s1, imm2):
        v = in0.astype(np.float32)
        m = ((v + s0) * v + s1) * v + imm2
        return (m * m) * (m * m)

    body = sq(sq(((Src0 + C0) * Src0 + C1) * Src0 + C2))
    spec = Spec(body=body, reference=_ref_exp4)
    shas = {}
    for ver in ("v3", "v4"):
        try:
            tmp = DveOpSpec(
                name="EXP4_POLY_ANT",
                opcode=len(dve_ops.OPS) + 1,
                uops=lower(spec, ver=ver),
                rd1_en=False,
            )
            shas[ver] = tmp.sha(ver)
        except Exception:
            pass
    op = DveOp("EXP4_POLY_ANT", spec, subdim=False, uops_sha=shas)
    dve_ops.OPS.append(op)
    dve_ops._SUB_OPCODE_FOR_NAME[op.name] = len(dve_ops.OPS)
    dve_ops.CUSTOM_DVE_SPECS[op.name] = spec
    _EXP4_OP = op
    return op


def _split512(w):
    """split [0, w) into chunks of <=512"""
    out = []
    lo = 0
    while lo < w:
        hi = min(lo + 512, w)
        out.append((lo, hi))
        lo = hi
    return out


class _Chunk:
    __slots__ = ("h", "kb", "jt", "q0", "tq0", "w", "diag", "sc", "pt", "off", "mate")

    def __init__(self, h, kb, jt):
        self.h, self.kb, self.jt = h, kb, jt
        self.q0 = kb * P
        self.tq0 = max(self.q0, 1024 * jt)
        self.w = 1024 * (jt + 1) - self.tq0
        self.diag = self.tq0 == self.q0
        self.off = 0  # column offset inside the (possibly shared) sc tile
        self.mate = None  # pair predecessor when this is a PAIR_SECOND chunk


def build_program():
    exp4 = _register_exp4()
    divf = _register_divf()
    nc = bacc.Bacc("TRN2", target_bir_lowering=False, debug=False)

    qt_d = nc.dram_tensor("QT", [DM, S], MM, kind="ExternalInput").ap()
    kt_d = nc.dram_tensor("KT", [DM, S], MM, kind="ExternalInput").ap()
    vt_d = nc.dram_tensor("VT", [DM, S], MM, kind="ExternalInput").ap()
    # host pre-transposed: [p, which(k/q), j, col]
    wkq_d = nc.dram_tensor("WKQ", [P, 2, DM // P, COLS], MM, kind="ExternalInput").ap()
    wv_d = nc.dram_tensor("WV", [P, DM // P, COLS], MM, kind="ExternalInput").ap()
    wo_d = nc.dram_tensor("WO", [P, COLS // P, DM], MM, kind="ExternalInput").ap()
    im_d = nc.dram_tensor("IM", [P, 2 * P], MM, kind="ExternalInput").ap()
    bkq_d = nc.dram_tensor("BKQ", [P, 2, COLS // P], FP32, kind="ExternalInput").ap()
    out_d = nc.dram_tensor("OUT", [S, DM], MM, kind="ExternalOutput").ap()

    NJ = DM // P  # 4 dm slabs
    NT = S // 512  # 4 seq tiles

    with tile.TileContext(nc) as tc, ExitStack() as ctx:
        const = ctx.enter_context(tc.tile_pool(name="const", bufs=1))
        persist = ctx.enter_context(tc.tile_pool(name="persist", bufs=1))

        # ---- constants (gpsimd ring), host-pre-transposed so every DMA
        # is [128 rows x >=1KB contiguous] ----
        wkq_sb = const.tile([P, 2, NJ, COLS], MM, tag="wkq")
        nc.gpsimd.dma_start(out=wkq_sb[:], in_=wkq_d[:, :, :, :])
        bkq_sb = const.tile([P, 2, COLS // P], FP32, tag="bkq")
        nc.gpsimd.dma_start(out=bkq_sb[:], in_=bkq_d[:, :, :])
        im_sb = const.tile([P, 2 * P], MM, tag="im")
        nc.gpsimd.dma_start(out=im_sb[:], in_=im_d[:, :])
        idn_sb = im_sb[:, 0:P]
        msk_sb = im_sb[:, P : 2 * P]
        wv_sb = const.tile([P, NJ, COLS], MM, tag="wv")  # after vt_h1 below
        wo_sb = const.tile([P, COLS // P, DM], MM, tag="wo")
        # per-partition ln(g) bias for the ACT exp
        lng_sb = const.tile([P, 1], FP32, tag="lng")
        nc.vector.memset(lng_sb[:], LN_G)
        # scratch operand for the PE warm-up burst (no DMA dependency)
        warm_sb = const.tile([P, P], MM, tag="warm")
        nc.vector.memset(warm_sb[:], 0.125)

        # ---- persistent activations ----
        qt_st = [persist.tile([P, S], MM, tag=f"qst{i}", name=f"qst{i}") for i in range(2)]
        kt_st = [persist.tile([P, S], MM, tag=f"kst{i}", name=f"kst{i}") for i in range(2)]
        v_sb = persist.tile([P, NKB, HC, DK + 1], MM, tag="vaug")
        nc.vector.memset(v_sb[:, :, :, DK : DK + 1], 1.0)
        ctxt_sb = [persist.tile([P, S], MM, tag=f"ctxt{i}", name=f"ctxt{i}") for i in range(2)]
        # per-head q/k with the 64 head dims duplicated onto BOTH partition
        # halves. The duplication doubles the scores (folded into SCALE/8)
        # and, critically, keeps the score matmuls at full 128-row PE
        # activity: the HAM clock governor reads 64-row matmuls as a
        # half-idle array and clamps the PE to 1.2 GHz for ~23 windows at a
        # time (measured 16-on/23-off duty cycling, +30us on the kernel).
        qt_dup = [persist.tile([P, S], MM, tag=f"qtd{h}", name=f"qtd{h}") for h in range(HC)]
        kt_dup = [persist.tile([P, S], MM, tag=f"ktd{h}", name=f"ktd{h}") for h in range(HC)]

        # ================= Phase A: q/k projections =================
        xin = ctx.enter_context(tc.tile_pool(name="xin", bufs=4))
        kt_tiles = [xin.tile([P, S], MM, tag="x", bufs=4, name=f"kx{j}") for j in range(NJ)]
        qt_tiles = [xin.tile([P, S], MM, tag="xq", bufs=4, name=f"qx{j}") for j in range(NJ)]
        v_tiles = [xin.tile([P, S], MM, tag="xv", bufs=4, name=f"vt{j}") for j in range(NJ)]

        # sync ring: KT halves (low first -- smaller first transfers reach
        # the PE sooner on the shared HBM), then VT low half
        for hf in range(2):
            for j in range(NJ):
                nc.sync.dma_start(
                    out=kt_tiles[j][:, hf * 1024 : (hf + 1) * 1024],
                    in_=kt_d[j * P : (j + 1) * P, hf * 1024 : (hf + 1) * 1024],
                )
        for j in range(NJ):
            nc.sync.dma_start(
                out=v_tiles[j][:, 0:1024], in_=vt_d[j * P : (j + 1) * P, 0:1024]
            )
        # scalar ring: QT low half now; high half deferred below so KT/QT-lo
        # keep HBM priority
        for j in range(NJ):
            nc.scalar.dma_start(
                out=qt_tiles[j][:, 0:1024], in_=qt_d[j * P : (j + 1) * P, 0:1024]
            )
        # gpsimd ring: VT high half, then wv + wo (needed progressively later)
        for j in range(NJ):
            nc.gpsimd.dma_start(
                out=v_tiles[j][:, 1024:2048], in_=vt_d[j * P : (j + 1) * P, 1024:2048]
            )
        nc.gpsimd.dma_start(out=wv_sb[:], in_=wv_d[:, :, :])
        nc.gpsimd.dma_start(out=wo_sb[:], in_=wo_d[:, :, :])

        def dup(st, dst, h, q2):
            # duplicate the head's 64 dims onto both partition halves
            # for seq half q2 (sbuf->sbuf DMA; kt on gpsimd, qt on sync)
            eng = nc.gpsimd if st is kt_st else nc.sync
            qs = slice(q2 * (S // 2), (q2 + 1) * (S // 2))
            s_ap = st[h // 2][(h % 2) * DK : (h % 2) * DK + DK, qs]
            for half in range(2):
                eng.dma_start(
                    out=dst[h][half * DK : (half + 1) * DK, qs], in_=s_ap
                )

        def make_proj_group(pool, tag, bufs):
            def proj_group(which, t, cc, st, cb_engine):
                ps = pool.tile(
                    [P, 512], FP32, tag=tag, bufs=bufs, name=f"ps{which}{cc}_{t}"
                )
                for j in range(NJ):
                    nc.tensor.matmul(
                        ps[:],
                        wkq_sb[:, which, j, cc * P : (cc + 1) * P],
                        (kt_tiles if which == 0 else qt_tiles)[j][
                            :, t * 512 : (t + 1) * 512
                        ],
                        start=(j == 0),
                        stop=(j == NJ - 1),
                        skip_group_check=True,
                    )
                ts_ = slice(t * 512, (t + 1) * 512)
                if cb_engine == "act":
                    nc.scalar.activation(
                        st[cc][:, ts_],
                        ps[:],
                        AF.Identity,
                        bias=bkq_sb[:, which, cc : cc + 1],
                    )
                else:
                    nc.vector.tensor_scalar_add(
                        st[cc][:, ts_], ps[:], bkq_sb[:, which, cc : cc + 1]
                    )

            return proj_group

        # Phase A proper covers only seq halves t=0,1 (the low 1 MB each of
        # KT and QT): head 0's jt=0 chunks need nothing else, so the t=2,3
        # projection groups are woven into head 0's chunk stream below and
        # the PE starts attention ~7us earlier instead of idling on DMA.
        with tc.tile_pool(name="pj_psum", bufs=4, space="PSUM") as pj_psum:
            # PE warm-up: the HAM clock governor starts the PE at 1.2 GHz
            # and needs ~3.4us of sustained activity to release full clock.
            # The PE would otherwise idle here waiting for the first input
            # DMAs; dummy matmuls on the memset tile warm it for free.
            warm_ps = pj_psum.tile([P, P], FP32, tag="warm", bufs=1, name="warmps")
            for _ in range(40):
                nc.tensor.matmul(
                    warm_ps[:],
                    warm_sb[:],
                    warm_sb[:],
                    start=True,
                    stop=True,
                    skip_group_check=True,
                )
            proj_a = make_proj_group(pj_psum, "ps", 4)
            for t in range(2):
                for cc in range(2):
                    proj_a(0, t, cc, kt_st, "act")
                if t == 0:
                    # release QT high-half DMAs after the t=0 K groups are
                    # emitted: scalar engine is in-order, so these issue
                    # after the first copybacks, giving the low halves HBM
                    # priority.
                    for jj in range(NJ):
                        nc.scalar.dma_start(
                            out=qt_tiles[jj][:, 1024:2048],
                            in_=qt_d[jj * P : (jj + 1) * P, 1024:2048],
                        )
                for cc in range(2):
                    proj_a(1, t, cc, qt_st, "act")
            for h in range(HC):
                dup(kt_st, kt_dup, h, 0)
                dup(qt_st, qt_dup, h, 0)

        # ================= Phase B: pipelined attention =================
        with tc.tile_pool(name="pb_psum", bufs=2, space="PSUM") as pb, tc.tile_pool(
            name="pt", bufs=4
        ) as pt_pool, tc.tile_pool(name="norm", bufs=4) as norm_pool, tc.tile_pool(
            name="osb", bufs=4
        ) as osb:
            po_tiles = {}

            def get_po(h, qq):
                key = (h, qq)
                if key not in po_tiles:
                    po_tiles[key] = pb.tile(
                        [DK + 1, 512], FP32, tag="po", bufs=4, name=f"po{h}_{qq}"
                    )
                return po_tiles[key]

            def vproj(tb):
                # rides the po-tag PSUM ring so the sc ring stays
                # double-buffered for the scores pipeline
                ps = pb.tile([P, HC, DK], FP32, tag="po", bufs=4, name=f"vps{tb}")
                for j in range(NJ):
                    nc.tensor.matmul(
                        ps[:, :, :],
                        v_tiles[j][:, tb * P : (tb + 1) * P],
                        wv_sb[:, j, :],
                        start=(j == 0),
                        stop=(j == NJ - 1),
                        skip_group_check=True,
                    )
                # v bias folded into bo on the host (softmax weights sum to 1)
                nc.vector.tensor_copy(v_sb[:, tb, :, 0:DK], ps[:, :, :])

            def oproj(tb):
                ps = pb.tile([P, DM], FP32, tag="po", bufs=4, name=f"ops{tb}")
                for cc in range(COLS // P):
                    nc.tensor.matmul(
                        ps[:],
                        ctxt_sb[cc][:, tb * P : (tb + 1) * P],
                        wo_sb[:, cc, :],
                        start=(cc == 0),
                        stop=(cc == COLS // P - 1),
                        skip_group_check=True,
                    )
                o = osb.tile([P, DM], MM, tag="o", name=f"o{tb}")
                if tb % 2:  # alternate the copy engines
                    nc.scalar.copy(o[:], ps[:])
                else:
                    nc.vector.tensor_copy(o[:], ps[:])
                # alternate output DMA rings too (drains in parallel)
                eng = nc.gpsimd if tb % 2 else nc.sync
                eng.dma_start(out=out_d[tb * P : (tb + 1) * P, :], in_=o[:])

            def normalize_cols(h, qq, off, width, done):
                """ctxT[head, cols] = po[0:64] / po[64] for `width` columns
                at quarter offset `off`: stage the sums row to partition 0,
                broadcast the RAW sums, then one fused approx-divide reading
                PSUM directly (recip seed + 1 Newton pass, ~0.17% max err)."""
                ti, po_ = h // 2, (h % 2) * DK
                po_q = get_po(h, qq)
                nm = f"{h}_{qq}_{off}"
                sums = norm_pool.tile([1, 512], FP32, tag="sums", name=f"s{nm}")
                nc.vector.tensor_copy(sums[:, :width], po_q[DK : DK + 1, off : off + width])
                bcast = norm_pool.tile([DK, 512], FP32, tag="bcast", name=f"b{nm}")
                nc.gpsimd.partition_broadcast(bcast[:, :width], sums[:, :width])
                g0 = qq * 512 + off
                if po_ == 0:
                    dst = ctxt_sb[ti][0:DK, g0 : g0 + width]
                else:
                    # custom-DVE ops cannot WRITE across partition bases
                    # (lanes are base-locked; measured garbage at base 64):
                    # stage at base 0 and move with a standard copy, which
                    # does handle the base change.
                    stage = norm_pool.tile([DK, 512], MM, tag="stg", name=f"g{nm}")
                    dst = stage[:, :width]
                nc.vector._custom_dve(
                    divf,
                    out=dst,
                    in0=po_q[0:DK, off : off + width],
                    in1=bcast[:, :width],
                    s0=DIV_C0,
                    s1=DIV_C1,
                )
                if po_ != 0:
                    nc.vector.tensor_copy(
                        ctxt_sb[ti][po_ : po_ + DK, g0 : g0 + width], dst
                    )
                if done:
                    del po_tiles[(h, qq)]

            def emit_scores(c):
                if c.mate is not None:
                    c.sc = c.mate.sc
                    c.off = c.mate.off + c.mate.w
                else:
                    c.sc = pb.tile(
                        [P, 1024], FP32, tag="sc", bufs=2, name=f"sc{c.h}_{c.kb}_{c.jt}"
                    )
                kt_h, qt_h = kt_dup[c.h], qt_dup[c.h]
                for lo, hi in _split512(c.w):
                    nc.tensor.matmul(
                        c.sc[:, c.off + lo : c.off + hi],
                        kt_h[:, c.q0 : c.q0 + P],
                        qt_h[:, c.tq0 + lo : c.tq0 + hi],
                        start=True,
                        stop=not (c.diag and lo == 0),
                        skip_group_check=True,
                    )
                if c.diag:  # add -30000 above the diagonal of the first block
                    nc.tensor.matmul(
                        c.sc[:, c.off : c.off + P],
                        idn_sb,
                        msk_sb,
                        start=False,
                        stop=True,
                        skip_group_check=True,
                    )

            def act_exp(c, lo, hi):
                nc.scalar.activation(
                    c.pt[:, lo:hi],
                    c.sc[:, lo:hi],
                    AF.Exp,
                    scale=4.0,
                    bias=lng_sb[:, 0:1],
                )

            def dve_exp(c, lo, hi):
                nc.vector._custom_dve(
                    exp4,
                    out=c.pt[:, lo:hi],
                    in0=c.sc[:, lo:hi],
                    s0=PA,
                    s1=PB,
                    imm2=PC,
                )

            def emit_exp(c):
                # for a pair, one exp covers both chunks' column ranges
                first = c.mate if c.mate is not None else c
                lo, hi = first.off, c.off + c.w
                c.pt = pt_pool.tile(
                    [P, 1024], MM, tag="pt", name=f"pt{c.h}_{c.kb}_{c.jt}"
                )
                if c.mate is not None:
                    c.mate.pt = c.pt
                if c.jt == 1 and c.kb in KB_DVE:
                    dve_exp(c, lo, hi)  # off-diagonal chunk: poly on the DVE
                elif (c.jt, c.kb) in SPLIT_DVE:
                    act_exp(c, lo, lo + P)  # masked diag block: hw exp -> 0
                    dve_exp(c, lo + P, hi)
                else:
                    act_exp(c, lo, hi)

            def pv_range(c, r0, r1):
                qc = (r0 // 512) * 512
                while qc < r1:
                    glo, ghi = max(r0, qc), min(r1, qc + 512)
                    qq = qc // 512
                    po_q = get_po(c.h, qq)
                    nc.tensor.matmul(
                        po_q[:, glo - qq * 512 : ghi - qq * 512],
                        v_sb[:, c.kb, c.h, :],
                        c.pt[:, c.off + glo - c.tq0 : c.off + ghi - c.tq0],
                        start=(c.kb == 0),
                        stop=(c.kb == LAST_KB[qq]),
                        skip_group_check=True,
                    )
                    qc += 512

            def emit_pv(c):
                if (c.jt, c.kb) in SPLIT_DVE:
                    # split at the exp engine boundary: the 128-col diag
                    # piece waits only on ACT, the tail only on the DVE
                    pv_range(c, c.tq0, c.tq0 + P)
                    pv_range(c, c.tq0 + P, c.tq0 + c.w)
                else:
                    pv_range(c, c.tq0, c.tq0 + c.w)

            # chunk stream: per head all jt=0 (low q) first, then jt=1
            chunks = []
            for h in range(HC):
                for kb in range(8):
                    chunks.append(_Chunk(h, kb, 0))
                for kb in range(NKB):
                    chunks.append(_Chunk(h, kb, 1))
            for i, c in enumerate(chunks):
                if (c.jt, c.kb) in PAIR_SECOND:
                    pj, pkb = PAIR_SECOND[(c.jt, c.kb)]
                    prev = chunks[i - 1]
                    assert (prev.jt, prev.kb) == (pj, pkb) and prev.h == c.h
                    c.mate = prev

            # the deferred t=2,3 projection groups, woven into head 0's
            # chunk stream (copybacks on the DVE: ACT is running h0's exps).
            # cc=0 first: head 0/1's high-q dups must be ready by the time
            # h0's jt=1 chunks start at slot 8.
            proj_b = make_proj_group(pb, "po", 4)
            deferred = [
                (which, t, cc)
                for cc in (0, 1)
                for which in (0, 1)
                for t in (2, 3)
            ]

            def pre_actions(i, c):
                # spread the late V-projection tiles across h0 (1 per 2 steps)
                if c.h == 0 and i % 2 == 0 and 4 + i // 2 < NKB:
                    vproj(4 + i // 2)
                # weave one deferred projection group into every other h0 slot
                if c.h == 0 and i % 2 == 1 and deferred:
                    which, t, cc = deferred.pop(0)
                    proj_b(which, t, cc, kt_st if which == 0 else qt_st, "dve")
                    if len(deferred) in (0, 4):  # one cc fully projected:
                        for h in (0, 1) if len(deferred) == 4 else (2, 3):
                            dup(kt_st, kt_dup, h, 1)
                            dup(qt_st, qt_dup, h, 1)

            def post_actions(c):
                h, kb, jt = c.h, c.kb, c.jt
                last = h == HC - 1
                if jt == 0 and kb == 3:
                    normalize_cols(h, 0, 0, 512, True)
                    if last:  # each oproj quad as soon as its quarter is done
                        for tb in range(0, 4):
                            oproj(tb)
                elif jt == 0 and kb == 7:
                    normalize_cols(h, 1, 0, 512, True)
                    if last:
                        for tb in range(4, 8):
                            oproj(tb)
                elif jt == 1 and kb == 11:
                    normalize_cols(h, 2, 0, 512, True)
                    if last:
                        for tb in range(8, 12):
                            oproj(tb)
                elif jt == 1 and kb == 15:
                    # drain: normalize the final quarter in 256-col pieces,
                    # each followed by its two output-projection tiles
                    normalize_cols(h, 3, 0, 256, False)
                    if last:
                        oproj(12)
                        oproj(13)
                    normalize_cols(h, 3, 256, 256, True)
                    if last:
                        oproj(14)
                        oproj(15)

            # V projection for key blocks 0-3 must precede the first PVs;
            # the rest stream in via pre_actions
            for tb in range(4):
                vproj(tb)

            # software-pipelined emission: scores of chunk i+1 land on the
            # PE queue before PV of chunk i, hiding the exp latency.
            # Pair-first chunks defer their exp+PV to the pair-second's
            # slot, where one merged exp covers both column ranges.
            is_first = {id(c.mate) for c in chunks if c.mate is not None}
            emit_scores(chunks[0])
            pre_actions(0, chunks[0])
            for i, c in enumerate(chunks):
                if i + 1 < len(chunks):
                    emit_scores(chunks[i + 1])
                    pre_actions(i + 1, chunks[i + 1])
                if id(c) in is_first:
                    continue
                emit_exp(c)
                if c.mate is not None:
                    emit_pv(c.mate)
                emit_pv(c)
                post_actions(c)

    nc.compile()
    return nc


def _get_nc():
    global _CACHED_NC
    if _CACHED_NC is None:
        _CACHED_NC = build_program()
    return _CACHED_NC


def make_in_maps(Q, K, V, Wq, bq, Wk, bk, Wv, bv, Wo, bo):
    f32 = lambda a: np.ascontiguousarray(a, dtype=np.float32)
    mm = lambda a: np.ascontiguousarray(np.asarray(a), dtype=MM_NP)
    qt = [mm(np.asarray(Q[b]).T) for b in range(B)]
    kt = [mm(np.asarray(K[b]).T) for b in range(B)]
    vt = [mm(np.asarray(V[b]).T) for b in range(B)]
    im = np.concatenate(
        [
            np.eye(P, dtype=MM_NP),
            np.tril(np.full((P, P), -30000.0, dtype=np.float32), -1).astype(MM_NP),
        ],
        axis=1,
    )
    qsc = SCALE / 8.0  # folded (incl. the 2x from dup) so PSUM scores are v
    maps = []
    for c in range(8):
        b, hg = c % B, c // B
        cs = slice(hg * COLS, (hg + 1) * COLS)
        wk_cs = np.asarray(Wk[:, cs], dtype=np.float32)
        wq_cs = np.asarray(Wq[:, cs], dtype=np.float32) * qsc
        wkq = (
            np.stack([wk_cs, wq_cs], axis=1)  # [512, 2, 256]
            .reshape(NJ_, P, 2, COLS)
            .transpose(1, 2, 0, 3)  # [128, 2, 4, 256]
        )
        wv_cs = np.asarray(Wv[:, cs], dtype=np.float32).reshape(NJ_, P, COLS).transpose(1, 0, 2)
        wo_cs = np.asarray(Wo[cs, :], dtype=np.float32).reshape(2, P, DM).transpose(1, 0, 2)
        bkq = np.stack(
            [
                f32(bk[cs]).reshape(2, P).T,
                (f32(bq[cs]) * qsc).reshape(2, P).T,
            ],
            axis=1,
        )  # [128, 2, 2]
        maps.append(
            {
                "QT": qt[b],
                "KT": kt[b],
                "VT": vt[b],
                "WKQ": mm(wkq),
                "WV": mm(wv_cs),
                "WO": mm(wo_cs),
                "IM": im,
                "BKQ": np.ascontiguousarray(bkq, dtype=np.float32),
            }
        )
    return maps


NJ_ = DM // P


def assemble(results, bv, Wo, bo):
    out = np.empty((B, S, DM), dtype=np.float32)
    for b in range(B):
        out[b] = results[b]["OUT"].astype(np.float32) + results[b + B][
            "OUT"
        ].astype(np.float32)
    # bv passes through attention unchanged (softmax rows sum to 1)
    bo_eff = np.asarray(bo, dtype=np.float32) + np.asarray(bv, np.float32) @ np.asarray(
        Wo, np.float32
    )
    out += bo_eff
    return out


def kernel(Q, K, V, Wq, bq, Wk, bk, Wv, bv, Wo, bo):
    nc = _get_nc()
    maps = make_in_maps(Q, K, V, Wq, bq, Wk, bk, Wv, bv, Wo, bo)
    res = run_bass_kernel_spmd(nc, maps, list(range(8)))
    return assemble(res.results, bv, Wo, bo)


# revision 43
# speedup vs baseline: 1.0536x; 1.0536x over previous
"""Multi-head causal attention on 8 Trainium2 NeuronCores.

Problem: B=4, S=2048, d_model=512, H=8 heads, d_k=64, fp32, causal,
scale = 1/sqrt(d_model) (faithful source quirk).

Sharding: 32 (batch, head-group) units -> core c handles batch c%4 and
head group c//4 (4 heads = 256 projection columns). Each core computes
q/k/v projections for its column slice, causal attention for its 4
heads, and a partial output projection (its 256 rows of Wo). The host
sums the two partials per batch and adds the output bias (with bv@Wo
pre-folded into it -- softmax weights sum to 1, so the V bias passes
through attention unchanged).

Phase A streams inputs on three DMA rings at once (sync: KT + low VT,
scalar: QT, gpsimd: pre-transposed weights + high VT) in half-slab
granularity, and emits projection groups t-major so the PE starts as
soon as the first half-slabs land. K copybacks run on ACT (Identity
with per-partition bias), Q copybacks on the DVE, in parallel.

Scores use per-head q/k duplicated onto both partition halves (the 2x
is folded into SCALE/8 on the host): matmul cost is per streamed
column either way, but 64-row matmuls read as a half-idle array to the
HAM clock governor, which then duty-cycles the PE at 1.2 GHz (measured
16 windows full clock / 23 windows half clock, +30us). The 128-row form
keeps the PE at 2.4 GHz.

Phase B is software-pipelined: the scores matmul for chunk i+1 is
emitted on the PE BEFORE the PV matmul of chunk i, so the PE computes
the next block's scores while the exp of the current one runs. The exp
is split across TWO engines: the ACT engine (hardware exp(4v + ln g))
and the DVE, which evaluates exp(4v)*g as (((v+a)v+b)v+c)^4 -- a monic
cubic minimax fit of g^(1/4)*e^v on |v|<=0.78 -- in a single custom-DVE
instruction. The monic normalization constant g cancels in the softmax.

Per head the chunks run jt=0 first (q in [q0,1024), kb 0-7) then jt=1
(q in [1024,2048), kb 0-15), accumulating into four [65,512] PSUM
quarters. Normalize = reciprocal of the sums row -> partition broadcast
-> multiply straight from PSUM. For the last head the high-q quarters
normalize in 128-column pieces as each key block retires, with the
output projection tile for that q-block following immediately.
"""

import sys

sys.path.insert(0, "/opt/trn_rl_repo")

from contextlib import ExitStack

import numpy as np

import concourse.bass as bass
import concourse.tile as tile
from concourse import bacc, mybir
from concourse.bass_utils import run_bass_kernel_spmd

FP32 = mybir.dt.float32
FP16 = mybir.dt.float16
MM = FP16  # matmul operand dtype
MM_NP = np.float16
AF = mybir.ActivationFunctionType

B, S, DM, H = 4, 2048, 512, 8
DK = DM // H  # 64
HC = 4  # heads per core
COLS = HC * DK  # 256
P = 128
NKB = S // P  # 16 key blocks
SCALE = 1.0 / float(np.sqrt(np.float32(DM)))
LAST_KB = (3, 7, 11, 15)  # last kb contributing to each q-quarter
# jt=1 chunks whose exp runs on the DVE, alternating with ACT chunks so
# neither exp engine starves the scores PSUM ring.
# OFF-DIAGONAL ONLY: the poly overflows on -30000 masked diag entries.
KB_DVE = (0, 2, 4, 6)
# diag chunks whose off-diagonal tail (cols 128:w) runs on the DVE; their
# PV is split at the same boundary so each PV piece gates on its own exp
# engine. Relieves ACT in the all-diag stretches (jt0 and kb8-15).
SPLIT_DVE = {(0, 1), (0, 2), (0, 3), (1, 9), (1, 11), (1, 13)}
# narrow chunks sharing one scores tile and ONE merged exp call with their
# predecessor -- halves the ACT per-call fixed cost (~293ns) where the
# chunks are too small to amortize it
PAIR_SECOND = {(0, 5): (0, 4), (0, 7): (0, 6)}

# monic cubic minimax fit of g^(1/4) * e^v on [-0.78, 0.78]:
# m(v) = ((v + PA)*v + PB)*v + PC,  m(v)^4 = g * e^(4v) (rel err < 8e-3)
PA = 3.243170435898654
PB = 6.2111458766350705
PC = 6.176377242985076
G = 1464.1623445051969
LN_G = float(np.log(G))

_CACHED_NC = None
_EXP4_OP = None
_DIVF_OP = None

# 1-NR approximate-divide constants (the Chebyshev pair from
# RECIP_APPROX_FAST is already 1-pass-optimal: max rel err ~0.17%)
DIV_C0 = -0.23549792
DIV_C1 = 2.0017324


def _register_divf():
    """Register DIV_APPROX_ANT: out = Src0 * recip_approx(Src1), where the
    reciprocal is a BITWISE_NOT seed + one inline Newton pass (6 slices).
    Replaces the separate reciprocal+multiply pair in softmax normalize."""
    global _DIVF_OP
    if _DIVF_OP is not None:
        return _DIVF_OP
    import concourse.dve_ops as dve_ops
    from concourse.dve_ops import DveOp
    from concourse.dve_spec import Spec, Src0, Src1, C0, C1, AluOp, Bin, lower
    from concourse.dve_uop import DveOpSpec

    for op in dve_ops.OPS:
        if op.name == "DIV_APPROX_ANT":
            _DIVF_OP = op
            return op

    def _ref_divf(in0, in1, s0, s1, imm2):
        not_d = (~in1.view(np.int32)).view(np.float32)
        y0 = not_d * np.float32(s0)
        y1 = y0 * (np.float32(s1) - in1 * y0)
        return in0 * y1

    _nd = Bin(AluOp.BITWISE_NOT, Src1, Src1)
    _z0 = _nd * C0
    _z1 = _z0 * (C1 - Src1 * _z0)
    spec = Spec(body=Src0 * _z1, reference=_ref_divf)
    shas = {}
    for ver in ("v3", "v4"):
        try:
            tmp = DveOpSpec(
                name="DIV_APPROX_ANT",
                opcode=len(dve_ops.OPS) + 1,
                uops=lower(spec, ver=ver),
                rd1_en=True,
            )
            shas[ver] = tmp.sha(ver)
        except Exception:
            pass
    op = DveOp("DIV_APPROX_ANT", spec, subdim=False, uops_sha=shas)
    dve_ops.OPS.append(op)
    dve_ops._SUB_OPCODE_FOR_NAME[op.name] = len(dve_ops.OPS)
    dve_ops.CUSTOM_DVE_SPECS[op.name] = spec
    _DIVF_OP = op
    return op


def _register_exp4():
    """Register the custom-DVE op EXP4_POLY_ANT (idempotent)."""
    global _EXP4_OP
    if _EXP4_OP is not None:
        return _EXP4_OP
    import concourse.dve_ops as dve_ops
    from concourse.dve_ops import DveOp
    from concourse.dve_spec import Spec, Src0, C0, C1, C2, lower, sq
    from concourse.dve_uop import DveOpSpec

    for op in dve_ops.OPS:
        if op.name == "EXP4_POLY_ANT":
            _EXP4_OP = op
            return op

    def _ref_exp4(in0, in1, s0, s1, imm2):
        v = in0.astype(np.float32)
        m = ((v + s0) * v + s1) * v + imm2
        return (m * m) * (m * m)

    body = sq(sq(((Src0 + C0) * Src0 + C1) * Src0 + C2))
    spec = Spec(body=body, reference=_ref_exp4)
    shas = {}
    for ver in ("v3", "v4"):
        try:
            tmp = DveOpSpec(
                name="EXP4_POLY_ANT",
                opcode=len(dve_ops.OPS) + 1,
                uops=lower(spec, ver=ver),
                rd1_en=False,
            )
            shas[ver] = tmp.sha(ver)
        except Exception:
            pass
    op = DveOp("EXP4_POLY_ANT", spec, subdim=False, uops_sha=shas)
    dve_ops.OPS.append(op)
    dve_ops._SUB_OPCODE_FOR_NAME[op.name] = len(dve_ops.OPS)
    dve_ops.CUSTOM_DVE_SPECS[op.name] = spec
    _EXP4_OP = op
    return op


def _split512(w):
    """split [0, w) into chunks of <=512"""
    out = []
    lo = 0
    while lo < w:
        hi = min(lo + 512, w)
        out.append((lo, hi))
        lo = hi
    return out


class _Chunk:
    __slots__ = ("h", "kb", "jt", "q0", "tq0", "w", "diag", "sc", "pt", "off", "mate")

    def __init__(self, h, kb, jt):
        self.h, self.kb, self.jt = h, kb, jt
        self.q0 = kb * P
        self.tq0 = max(self.q0, 1024 * jt)
        self.w = 1024 * (jt + 1) - self.tq0
        self.diag = self.tq0 == self.q0
        self.off = 0  # column offset inside the (possibly shared) sc tile
        self.mate = None  # pair predecessor when this is a PAIR_SECOND chunk


def build_program():
    exp4 = _register_exp4()
    divf = _register_divf()
    nc = bacc.Bacc("TRN2", target_bir_lowering=False, debug=False)

    qt_d = nc.dram_tensor("QT", [DM, S], MM, kind="ExternalInput").ap()
    kt_d = nc.dram_tensor("KT", [DM, S], MM, kind="ExternalInput").ap()
    vt_d = nc.dram_tensor("VT", [DM, S], MM, kind="ExternalInput").ap()
    # host pre-transposed: [p, which(k/q), j, col]
    wkq_d = nc.dram_tensor("WKQ", [P, 2, DM // P, COLS], MM, kind="ExternalInput").ap()
    wv_d = nc.dram_tensor("WV", [P, DM // P, COLS], MM, kind="ExternalInput").ap()
    wo_d = nc.dram_tensor("WO", [P, COLS // P, DM], MM, kind="ExternalInput").ap()
    im_d = nc.dram_tensor("IM", [P, 2 * P], MM, kind="ExternalInput").ap()
    bkq_d = nc.dram_tensor("BKQ", [P, 2, COLS // P], FP32, kind="ExternalInput").ap()
    out_d = nc.dram_tensor("OUT", [S, DM], MM, kind="ExternalOutput").ap()

    NJ = DM // P  # 4 dm slabs
    NT = S // 512  # 4 seq tiles

    with tile.TileContext(nc) as tc, ExitStack() as ctx:
        const = ctx.enter_context(tc.tile_pool(name="const", bufs=1))
        persist = ctx.enter_context(tc.tile_pool(name="persist", bufs=1))

        # ---- constants (gpsimd ring), host-pre-transposed so every DMA
        # is [128 rows x >=1KB contiguous] ----
        wkq_sb = const.tile([P, 2, NJ, COLS], MM, tag="wkq")
        nc.gpsimd.dma_start(out=wkq_sb[:], in_=wkq_d[:, :, :, :])
        bkq_sb = const.tile([P, 2, COLS // P], FP32, tag="bkq")
        nc.gpsimd.dma_start(out=bkq_sb[:], in_=bkq_d[:, :, :])
        im_sb = const.tile([P, 2 * P], MM, tag="im")
        nc.gpsimd.dma_start(out=im_sb[:], in_=im_d[:, :])
        idn_sb = im_sb[:, 0:P]
        msk_sb = im_sb[:, P : 2 * P]
        wv_sb = const.tile([P, NJ, COLS], MM, tag="wv")  # after vt_h1 below
        wo_sb = const.tile([P, COLS // P, DM], MM, tag="wo")
        # per-partition ln(g) bias for the ACT exp
        lng_sb = const.tile([P, 1], FP32, tag="lng")
        nc.vector.memset(lng_sb[:], LN_G)

        # ---- persistent activations ----
        qt_st = [persist.tile([P, S], MM, tag=f"qst{i}", name=f"qst{i}") for i in range(2)]
        kt_st = [persist.tile([P, S], MM, tag=f"kst{i}", name=f"kst{i}") for i in range(2)]
        v_sb = persist.tile([P, NKB, HC, DK + 1], MM, tag="vaug")
        nc.vector.memset(v_sb[:, :, :, DK : DK + 1], 1.0)
        ctxt_sb = [persist.tile([P, S], MM, tag=f"ctxt{i}", name=f"ctxt{i}") for i in range(2)]
        # per-head q/k with the 64 head dims duplicated onto BOTH partition
        # halves. The duplication doubles the scores (folded into SCALE/8)
        # and, critically, keeps the score matmuls at full 128-row PE
        # activity: the HAM clock governor reads 64-row matmuls as a
        # half-idle array and clamps the PE to 1.2 GHz for ~23 windows at a
        # time (measured 16-on/23-off duty cycling, +30us on the kernel).
        qt_dup = [persist.tile([P, S], MM, tag=f"qtd{h}", name=f"qtd{h}") for h in range(HC)]
        kt_dup = [persist.tile([P, S], MM, tag=f"ktd{h}", name=f"ktd{h}") for h in range(HC)]

        # ================= Phase A: q/k projections =================
        xin = ctx.enter_context(tc.tile_pool(name="xin", bufs=4))
        kt_tiles = [xin.tile([P, S], MM, tag="x", bufs=4, name=f"kx{j}") for j in range(NJ)]
        qt_tiles = [xin.tile([P, S], MM, tag="xq", bufs=4, name=f"qx{j}") for j in range(NJ)]
        v_tiles = [xin.tile([P, S], MM, tag="xv", bufs=4, name=f"vt{j}") for j in range(NJ)]

        # sync ring: KT halves (low first -- smaller first transfers reach
        # the PE sooner on the shared HBM), then VT low half
        for hf in range(2):
            for j in range(NJ):
                nc.sync.dma_start(
                    out=kt_tiles[j][:, hf * 1024 : (hf + 1) * 1024],
                    in_=kt_d[j * P : (j + 1) * P, hf * 1024 : (hf + 1) * 1024],
                )
        for j in range(NJ):
            nc.sync.dma_start(
                out=v_tiles[j][:, 0:1024], in_=vt_d[j * P : (j + 1) * P, 0:1024]
            )
        # scalar ring: QT low half now; high half deferred below so KT/QT-lo
        # keep HBM priority
        for j in range(NJ):
            nc.scalar.dma_start(
                out=qt_tiles[j][:, 0:1024], in_=qt_d[j * P : (j + 1) * P, 0:1024]
            )
        # gpsimd ring: VT high half, then wv + wo (needed progressively later)
        for j in range(NJ):
            nc.gpsimd.dma_start(
                out=v_tiles[j][:, 1024:2048], in_=vt_d[j * P : (j + 1) * P, 1024:2048]
            )
        nc.gpsimd.dma_start(out=wv_sb[:], in_=wv_d[:, :, :])
        nc.gpsimd.dma_start(out=wo_sb[:], in_=wo_d[:, :, :])

        def dup(st, dst, h, q2):
            # duplicate the head's 64 dims onto both partition halves
            # for seq half q2 (sbuf->sbuf DMA; kt on gpsimd, qt on sync)
            eng = nc.gpsimd if st is kt_st else nc.sync
            qs = slice(q2 * (S // 2), (q2 + 1) * (S // 2))
            s_ap = st[h // 2][(h % 2) * DK : (h % 2) * DK + DK, qs]
            for half in range(2):
                eng.dma_start(
                    out=dst[h][half * DK : (half + 1) * DK, qs], in_=s_ap
                )

        def make_proj_group(pool, tag, bufs):
            def proj_group(which, t, cc, st, cb_engine):
                ps = pool.tile(
                    [P, 512], FP32, tag=tag, bufs=bufs, name=f"ps{which}{cc}_{t}"
                )
                for j in range(NJ):
                    nc.tensor.matmul(
                        ps[:],
                        wkq_sb[:, which, j, cc * P : (cc + 1) * P],
                        (kt_tiles if which == 0 else qt_tiles)[j][
                            :, t * 512 : (t + 1) * 512
                        ],
                        start=(j == 0),
                        stop=(j == NJ - 1),
                        skip_group_check=True,
                    )
                ts_ = slice(t * 512, (t + 1) * 512)
                if cb_engine == "act":
                    nc.scalar.activation(
                        st[cc][:, ts_],
                        ps[:],
                        AF.Identity,
                        bias=bkq_sb[:, which, cc : cc + 1],
                    )
                else:
                    nc.vector.tensor_scalar_add(
                        st[cc][:, ts_], ps[:], bkq_sb[:, which, cc : cc + 1]
                    )

            return proj_group

        with tc.tile_pool(name="pj_psum", bufs=8, space="PSUM") as pj_psum:
            proj_a = make_proj_group(pj_psum, "ps", 8)
            for t in range(NT):
                for cc in range(2):
                    proj_a(0, t, cc, kt_st, "act")
                if t == 0:
                    # release QT high-half DMAs after the t=0 K groups are
                    # emitted: scalar engine is in-order, so these issue
                    # after the first copybacks, giving the low halves HBM
                    # priority.
                    for jj in range(NJ):
                        nc.scalar.dma_start(
                            out=qt_tiles[jj][:, 1024:2048],
                            in_=qt_d[jj * P : (jj + 1) * P, 1024:2048],
                        )
                for cc in range(2):
                    proj_a(1, t, cc, qt_st, "act")
                if t == 1 or t == 3:
                    q2 = t // 2
                    for h in range(HC):
                        dup(kt_st, kt_dup, h, q2)
                        dup(qt_st, qt_dup, h, q2)

        # ================= Phase B: pipelined attention =================
        with tc.tile_pool(name="pb_psum", bufs=2, space="PSUM") as pb, tc.tile_pool(
            name="pt", bufs=4
        ) as pt_pool, tc.tile_pool(name="norm", bufs=4) as norm_pool, tc.tile_pool(
            name="osb", bufs=4
        ) as osb:
            po_tiles = {}

            def get_po(h, qq):
                key = (h, qq)
                if key not in po_tiles:
                    po_tiles[key] = pb.tile(
                        [DK + 1, 512], FP32, tag="po", bufs=4, name=f"po{h}_{qq}"
                    )
                return po_tiles[key]

            def vproj(tb):
                # rides the po-tag PSUM ring so the sc ring stays
                # double-buffered for the scores pipeline
                ps = pb.tile([P, HC, DK], FP32, tag="po", bufs=4, name=f"vps{tb}")
                for j in range(NJ):
                    nc.tensor.matmul(
                        ps[:, :, :],
                        v_tiles[j][:, tb * P : (tb + 1) * P],
                        wv_sb[:, j, :],
                        start=(j == 0),
                        stop=(j == NJ - 1),
                        skip_group_check=True,
                    )
                # v bias folded into bo on the host (softmax weights sum to 1)
                nc.vector.tensor_copy(v_sb[:, tb, :, 0:DK], ps[:, :, :])

            def oproj(tb):
                ps = pb.tile([P, DM], FP32, tag="po", bufs=4, name=f"ops{tb}")
                for cc in range(COLS // P):
                    nc.tensor.matmul(
                        ps[:],
                        ctxt_sb[cc][:, tb * P : (tb + 1) * P],
                        wo_sb[:, cc, :],
                        start=(cc == 0),
                        stop=(cc == COLS // P - 1),
                        skip_group_check=True,
                    )
                o = osb.tile([P, DM], MM, tag="o", name=f"o{tb}")
                if tb % 2:  # alternate the copy engines
                    nc.scalar.copy(o[:], ps[:])
                else:
                    nc.vector.tensor_copy(o[:], ps[:])
                # alternate output DMA rings too (drains in parallel)
                eng = nc.gpsimd if tb % 2 else nc.sync
                eng.dma_start(out=out_d[tb * P : (tb + 1) * P, :], in_=o[:])

            def normalize_cols(h, qq, off, width, done):
                """ctxT[head, cols] = po[0:64] / po[64] for `width` columns
                at quarter offset `off`: stage the sums row to partition 0,
                broadcast the RAW sums, then one fused approx-divide reading
                PSUM directly (recip seed + 1 Newton pass, ~0.17% max err)."""
                ti, po_ = h // 2, (h % 2) * DK
                po_q = get_po(h, qq)
                nm = f"{h}_{qq}_{off}"
                sums = norm_pool.tile([1, 512], FP32, tag="sums", name=f"s{nm}")
                nc.vector.tensor_copy(sums[:, :width], po_q[DK : DK + 1, off : off + width])
                bcast = norm_pool.tile([DK, 512], FP32, tag="bcast", name=f"b{nm}")
                nc.gpsimd.partition_broadcast(bcast[:, :width], sums[:, :width])
                g0 = qq * 512 + off
                if po_ == 0:
                    dst = ctxt_sb[ti][0:DK, g0 : g0 + width]
                else:
                    # custom-DVE ops cannot WRITE across partition bases
                    # (lanes are base-locked; measured garbage at base 64):
                    # stage at base 0 and move with a standard copy, which
                    # does handle the base change.
                    stage = norm_pool.tile([DK, 512], MM, tag="stg", name=f"g{nm}")
                    dst = stage[:, :width]
                nc.vector._custom_dve(
                    divf,
                    out=dst,
                    in0=po_q[0:DK, off : off + width],
                    in1=bcast[:, :width],
                    s0=DIV_C0,
                    s1=DIV_C1,
                )
                if po_ != 0:
                    nc.vector.tensor_copy(
                        ctxt_sb[ti][po_ : po_ + DK, g0 : g0 + width], dst
                    )
                if done:
                    del po_tiles[(h, qq)]

            def emit_scores(c):
                if c.mate is not None:
                    c.sc = c.mate.sc
                    c.off = c.mate.off + c.mate.w
                else:
                    c.sc = pb.tile(
                        [P, 1024], FP32, tag="sc", bufs=2, name=f"sc{c.h}_{c.kb}_{c.jt}"
                    )
                kt_h, qt_h = kt_dup[c.h], qt_dup[c.h]
                for lo, hi in _split512(c.w):
                    nc.tensor.matmul(
                        c.sc[:, c.off + lo : c.off + hi],
                        kt_h[:, c.q0 : c.q0 + P],
                        qt_h[:, c.tq0 + lo : c.tq0 + hi],
                        start=True,
                        stop=not (c.diag and lo == 0),
                        skip_group_check=True,
                    )
                if c.diag:  # add -30000 above the diagonal of the first block
                    nc.tensor.matmul(
                        c.sc[:, c.off : c.off + P],
                        idn_sb,
                        msk_sb,
                        start=False,
                        stop=True,
                        skip_group_check=True,
                    )

            def act_exp(c, lo, hi):
                nc.scalar.activation(
                    c.pt[:, lo:hi],
                    c.sc[:, lo:hi],
                    AF.Exp,
                    scale=4.0,
                    bias=lng_sb[:, 0:1],
                )

            def dve_exp(c, lo, hi):
                nc.vector._custom_dve(
                    exp4,
                    out=c.pt[:, lo:hi],
                    in0=c.sc[:, lo:hi],
                    s0=PA,
                    s1=PB,
                    imm2=PC,
                )

            def emit_exp(c):
                # for a pair, one exp covers both chunks' column ranges
                first = c.mate if c.mate is not None else c
                lo, hi = first.off, c.off + c.w
                c.pt = pt_pool.tile(
                    [P, 1024], MM, tag="pt", name=f"pt{c.h}_{c.kb}_{c.jt}"
                )
                if c.mate is not None:
                    c.mate.pt = c.pt
                if c.jt == 1 and c.kb in KB_DVE:
                    dve_exp(c, lo, hi)  # off-diagonal chunk: poly on the DVE
                elif (c.jt, c.kb) in SPLIT_DVE:
                    act_exp(c, lo, lo + P)  # masked diag block: hw exp -> 0
                    dve_exp(c, lo + P, hi)
                else:
                    act_exp(c, lo, hi)

            def pv_range(c, r0, r1):
                qc = (r0 // 512) * 512
                while qc < r1:
                    glo, ghi = max(r0, qc), min(r1, qc + 512)
                    qq = qc // 512
                    po_q = get_po(c.h, qq)
                    nc.tensor.matmul(
                        po_q[:, glo - qq * 512 : ghi - qq * 512],
                        v_sb[:, c.kb, c.h, :],
                        c.pt[:, c.off + glo - c.tq0 : c.off + ghi - c.tq0],
                        start=(c.kb == 0),
                        stop=(c.kb == LAST_KB[qq]),
                        skip_group_check=True,
                    )
                    qc += 512

            def emit_pv(c):
                if (c.jt, c.kb) in SPLIT_DVE:
                    # split at the exp engine boundary: the 128-col diag
                    # piece waits only on ACT, the tail only on the DVE
                    pv_range(c, c.tq0, c.tq0 + P)
                    pv_range(c, c.tq0 + P, c.tq0 + c.w)
                else:
                    pv_range(c, c.tq0, c.tq0 + c.w)

            # chunk stream: per head all jt=0 (low q) first, then jt=1
            chunks = []
            for h in range(HC):
                for kb in range(8):
                    chunks.append(_Chunk(h, kb, 0))
                for kb in range(NKB):
                    chunks.append(_Chunk(h, kb, 1))
            for i, c in enumerate(chunks):
                if (c.jt, c.kb) in PAIR_SECOND:
                    pj, pkb = PAIR_SECOND[(c.jt, c.kb)]
                    prev = chunks[i - 1]
                    assert (prev.jt, prev.kb) == (pj, pkb) and prev.h == c.h
                    c.mate = prev

            def pre_actions(i, c):
                # spread the late V-projection tiles across h0 (1 per 2 steps)
                if c.h == 0 and i % 2 == 0 and 4 + i // 2 < NKB:
                    vproj(4 + i // 2)

            def post_actions(c):
                h, kb, jt = c.h, c.kb, c.jt
                last = h == HC - 1
                if jt == 0 and kb == 3:
                    normalize_cols(h, 0, 0, 512, True)
                    if last:  # each oproj quad as soon as its quarter is done
                        for tb in range(0, 4):
                            oproj(tb)
                elif jt == 0 and kb == 7:
                    normalize_cols(h, 1, 0, 512, True)
                    if last:
                        for tb in range(4, 8):
                            oproj(tb)
                elif jt == 1 and kb == 11:
                    normalize_cols(h, 2, 0, 512, True)
                    if last:
                        for tb in range(8, 12):
                            oproj(tb)
                elif jt == 1 and kb == 15:
                    # drain: normalize the final quarter in 256-col pieces,
                    # each followed by its two output-projection tiles
                    normalize_cols(h, 3, 0, 256, False)
                    if last:
                        oproj(12)
                        oproj(13)
                    normalize_cols(h, 3, 256, 256, True)
                    if last:
                        oproj(14)
                        oproj(15)

            # V projection for key blocks 0-3 must precede the first PVs;
            # the rest stream in via pre_actions
            for tb in range(4):
                vproj(tb)

            # software-pipelined emission: scores of chunk i+1 land on the
            # PE queue before PV of chunk i, hiding the exp latency.
            # Pair-first chunks defer their exp+PV to the pair-second's
            # slot, where one merged exp covers both column ranges.
            is_first = {id(c.mate) for c in chunks if c.mate is not None}
            emit_scores(chunks[0])
            pre_actions(0, chunks[0])
            for i, c in enumerate(chunks):
                if i + 1 < len(chunks):
                    emit_scores(chunks[i + 1])
                    pre_actions(i + 1, chunks[i + 1])
                if id(c) in is_first:
                    continue
                emit_exp(c)
                if c.mate is not None:
                    emit_pv(c.mate)
                emit_pv(c)
                post_actions(c)

    nc.compile()
    return nc


def _get_nc():
    global _CACHED_NC
    if _CACHED_NC is None:
        _CACHED_NC = build_program()
    return _CACHED_NC


def make_in_maps(Q, K, V, Wq, bq, Wk, bk, Wv, bv, Wo, bo):
    f32 = lambda a: np.ascontiguousarray(a, dtype=np.float32)
    mm = lambda a: np.ascontiguousarray(np.asarray(a), dtype=MM_NP)
    qt = [mm(np.asarray(Q[b]).T) for b in range(B)]
    kt = [mm(np.asarray(K[b]).T) for b in range(B)]
    vt = [mm(np.asarray(V[b]).T) for b in range(B)]
    im = np.concatenate(
        [
            np.eye(P, dtype=MM_NP),
            np.tril(np.full((P, P), -30000.0, dtype=np.float32), -1).astype(MM_NP),
        ],
        axis=1,
    )
    qsc = SCALE / 8.0  # folded (incl. the 2x from dup) so PSUM scores are v
    maps = []
    for c in range(8):
        b, hg = c % B, c // B
        cs = slice(hg * COLS, (hg + 1) * COLS)
        wk_cs = np.asarray(Wk[:, cs], dtype=np.float32)
        wq_cs = np.asarray(Wq[:, cs], dtype=np.float32) * qsc
        wkq = (
            np.stack([wk_cs, wq_cs], axis=1)  # [512, 2, 256]
            .reshape(NJ_, P, 2, COLS)
            .transpose(1, 2, 0, 3)  # [128, 2, 4, 256]
        )
        wv_cs = np.asarray(Wv[:, cs], dtype=np.float32).reshape(NJ_, P, COLS).transpose(1, 0, 2)
        wo_cs = np.asarray(Wo[cs, :], dtype=np.float32).reshape(2, P, DM).transpose(1, 0, 2)
        bkq = np.stack(
            [
                f32(bk[cs]).reshape(2, P).T,
                (f32(bq[cs]) * qsc).reshape(2, P).T,
            ],
            axis=1,
        )  # [128, 2, 2]
        maps.append(
            {
                "QT": qt[b],
                "KT": kt[b],
                "VT": vt[b],
                "WKQ": mm(wkq),
                "WV": mm(wv_cs),
                "WO": mm(wo_cs),
                "IM": im,
                "BKQ": np.ascontiguousarray(bkq, dtype=np.float32),
            }
        )
    return maps


NJ_ = DM // P


def assemble(results, bv, Wo, bo):
    out = np.empty((B, S, DM), dtype=np.float32)
    for b in range(B):
        out[b] = results[b]["OUT"].astype(np.float32) + results[b + B][
            "OUT"
        ].astype(np.float32)
    # bv passes through attention unchanged (softmax rows sum to 1)
    bo_eff = np.asarray(bo, dtype=np.float32) + np.asarray(bv, np.float32) @ np.asarray(
        Wo, np.float32
    )
    out += bo_eff
    return out


def kernel(Q, K, V, Wq, bq, Wk, bk, Wv, bv, Wo, bo):
    nc = _get_nc()
    maps = make_in_maps(Q, K, V, Wq, bq, Wk, bk, Wv, bv, Wo, bo)
    res = run_bass_kernel_spmd(nc, maps, list(range(8)))
    return assemble(res.results, bv, Wo, bo)


# revision 46
# speedup vs baseline: 1.0811x; 1.0261x over previous
"""Multi-head causal attention on 8 Trainium2 NeuronCores.

Problem: B=4, S=2048, d_model=512, H=8 heads, d_k=64, fp32, causal,
scale = 1/sqrt(d_model) (faithful source quirk).

Sharding: 32 (batch, head-group) units -> core c handles batch c%4 and
head group c//4 (4 heads = 256 projection columns). Each core computes
q/k/v projections for its column slice, causal attention for its 4
heads, and a partial output projection (its 256 rows of Wo). The host
sums the two partials per batch and adds the output bias (with bv@Wo
pre-folded into it -- softmax weights sum to 1, so the V bias passes
through attention unchanged).

Phase A streams inputs on three DMA rings at once (sync: KT + low VT,
scalar: QT, gpsimd: pre-transposed weights + high VT) in half-slab
granularity, and emits projection groups t-major so the PE starts as
soon as the first half-slabs land. Copybacks run on ACT (Identity with
per-partition bias), which is otherwise idle through phase A.

Scores use per-head q/k duplicated onto both partition halves (the 2x
is folded into SCALE/8 on the host): matmul cost is per streamed
column either way, but 64-row matmuls read as a half-idle array to the
HAM clock governor, which then duty-cycles the PE at 1.2 GHz (measured
16 windows full clock / 23 windows half clock, +30us). The 128-row form
keeps the PE at 2.4 GHz.

Phase B is software-pipelined: the scores matmul for chunk i+1 is
emitted on the PE BEFORE the PV matmul of chunk i, so the PE computes
the next block's scores while the exp of the current one runs. The exp
is split across TWO engines: the ACT engine (hardware exp(4v + ln g))
and the DVE, which evaluates exp(4v)*g as (((v+a)v+b)v+c)^4 -- a monic
cubic minimax fit of g^(1/4)*e^v on |v|<=0.78 -- in a single custom-DVE
instruction. The monic normalization constant g cancels in the softmax.

Diag chunks split their exp at the 128-col mask boundary (ACT takes the
masked block, the DVE poly the off-diagonal tail) with the PV matmul
split at the same column so each piece gates on its own exp engine;
the narrow jt=0 tail chunks (kb 4+5, 6+7) share one scores tile and a
single merged exp to amortize ACT's 293ns fixed cost per call.

Per head the chunks run jt=0 first (q in [q0,1024), kb 0-7) then jt=1
(q in [1024,2048), kb 0-15), accumulating into four [65,512] PSUM
quarters. Normalize = stage the matmul-produced sums row to partition
0, gpsimd-broadcast the raw sums, then ONE fused custom-DVE
approximate divide (bitwise-NOT seed + 1 Newton pass, 0.17% max err)
reading the PSUM quarter directly; odd heads stage at base 0 first
(custom-DVE ops cannot write across partition bases). The last head's
output projections run as quads as soon as each quarter normalizes,
with the final quarter in 256-col pieces to pipeline the drain.
"""

import sys

sys.path.insert(0, "/opt/trn_rl_repo")

from contextlib import ExitStack

import numpy as np

import concourse.bass as bass
import concourse.tile as tile
from concourse import bacc, mybir
from concourse.bass_utils import run_bass_kernel_spmd

FP32 = mybir.dt.float32
FP16 = mybir.dt.float16
MM = FP16  # matmul operand dtype
MM_NP = np.float16
AF = mybir.ActivationFunctionType

B, S, DM, H = 4, 2048, 512, 8
DK = DM // H  # 64
HC = 4  # heads per core
COLS = HC * DK  # 256
P = 128
NKB = S // P  # 16 key blocks
SCALE = 1.0 / float(np.sqrt(np.float32(DM)))
LAST_KB = (3, 7, 11, 15)  # last kb contributing to each q-quarter
# jt=1 chunks whose exp runs on the DVE, alternating with ACT chunks so
# neither exp engine starves the scores PSUM ring.
# OFF-DIAGONAL ONLY: the poly overflows on -30000 masked diag entries.
KB_DVE = (0, 2, 3, 5, 7)
# diag-tail DVE splits: measured net-negative (48 extra exp calls + dual-
# engine PV gating stalls the kb8-15 stretch more than ACT relief helps)
SPLIT_DVE = set()
# narrow chunks sharing one scores tile and ONE merged exp call with their
# predecessor -- halves the ACT per-call fixed cost (~293ns) where the
# chunks are too small to amortize it
PAIR_SECOND = {(0, 5): (0, 4), (0, 7): (0, 6), (1, 13): (1, 12), (1, 15): (1, 14)}

# monic cubic minimax fit of g^(1/4) * e^v on [-0.78, 0.78]:
# m(v) = ((v + PA)*v + PB)*v + PC,  m(v)^4 = g * e^(4v) (rel err < 8e-3)
PA = 3.243170435898654
PB = 6.2111458766350705
PC = 6.176377242985076
G = 1464.1623445051969
LN_G = float(np.log(G))

_CACHED_NC = None
_EXP4_OP = None
_DIVF_OP = None

# 1-NR approximate-divide constants (the Chebyshev pair from
# RECIP_APPROX_FAST is already 1-pass-optimal: max rel err ~0.17%)
DIV_C0 = -0.23549792
DIV_C1 = 2.0017324


def _register_divf():
    """Register DIV_APPROX_ANT: out = Src0 * recip_approx(Src1), where the
    reciprocal is a BITWISE_NOT seed + one inline Newton pass (6 slices).
    Replaces the separate reciprocal+multiply pair in softmax normalize."""
    global _DIVF_OP
    if _DIVF_OP is not None:
        return _DIVF_OP
    import concourse.dve_ops as dve_ops
    from concourse.dve_ops import DveOp
    from concourse.dve_spec import Spec, Src0, Src1, C0, C1, AluOp, Bin, lower
    from concourse.dve_uop import DveOpSpec

    for op in dve_ops.OPS:
        if op.name == "DIV_APPROX_ANT":
            _DIVF_OP = op
            return op

    def _ref_divf(in0, in1, s0, s1, imm2):
        not_d = (~in1.view(np.int32)).view(np.float32)
        y0 = not_d * np.float32(s0)
        y1 = y0 * (np.float32(s1) - in1 * y0)
        return in0 * y1

    _nd = Bin(AluOp.BITWISE_NOT, Src1, Src1)
    _z0 = _nd * C0
    _z1 = _z0 * (C1 - Src1 * _z0)
    spec = Spec(body=Src0 * _z1, reference=_ref_divf)
    shas = {}
    for ver in ("v3", "v4"):
        try:
            tmp = DveOpSpec(
                name="DIV_APPROX_ANT",
                opcode=len(dve_ops.OPS) + 1,
                uops=lower(spec, ver=ver),
                rd1_en=True,
            )
            shas[ver] = tmp.sha(ver)
        except Exception:
            pass
    op = DveOp("DIV_APPROX_ANT", spec, subdim=False, uops_sha=shas)
    dve_ops.OPS.append(op)
    dve_ops._SUB_OPCODE_FOR_NAME[op.name] = len(dve_ops.OPS)
    dve_ops.CUSTOM_DVE_SPECS[op.name] = spec
    _DIVF_OP = op
    return op


def _register_exp4():
    """Register the custom-DVE op EXP4_POLY_ANT (idempotent)."""
    global _EXP4_OP
    if _EXP4_OP is not None:
        return _EXP4_OP
    import concourse.dve_ops as dve_ops
    from concourse.dve_ops import DveOp
    from concourse.dve_spec import Spec, Src0, C0, C1, C2, lower, sq
    from concourse.dve_uop import DveOpSpec

    for op in dve_ops.OPS:
        if op.name == "EXP4_POLY_ANT":
            _EXP4_OP = op
            return op

    def _ref_exp4(in0, in1, s0, s1, imm2):
        v = in0.astype(np.float32)
        m = ((v + s0) * v + s1) * v + imm2
        return (m * m) * (m * m)

    body = sq(sq(((Src0 + C0) * Src0 + C1) * Src0 + C2))
    spec = Spec(body=body, reference=_ref_exp4)
    shas = {}
    for ver in ("v3", "v4"):
        try:
            tmp = DveOpSpec(
                name="EXP4_POLY_ANT",
                opcode=len(dve_ops.OPS) + 1,
                uops=lower(spec, ver=ver),
                rd1_en=False,
            )
            shas[ver] = tmp.sha(ver)
        except Exception:
            pass
    op = DveOp("EXP4_POLY_ANT", spec, subdim=False, uops_sha=shas)
    dve_ops.OPS.append(op)
    dve_ops._SUB_OPCODE_FOR_NAME[op.name] = len(dve_ops.OPS)
    dve_ops.CUSTOM_DVE_SPECS[op.name] = spec
    _EXP4_OP = op
    return op


def _split512(w):
    """split [0, w) into chunks of <=512"""
    out = []
    lo = 0
    while lo < w:
        hi = min(lo + 512, w)
        out.append((lo, hi))
        lo = hi
    return out


class _Chunk:
    __slots__ = ("h", "kb", "jt", "q0", "tq0", "w", "diag", "sc", "pt", "off", "mate")

    def __init__(self, h, kb, jt):
        self.h, self.kb, self.jt = h, kb, jt
        self.q0 = kb * P
        self.tq0 = max(self.q0, 1024 * jt)
        self.w = 1024 * (jt + 1) - self.tq0
        self.diag = self.tq0 == self.q0
        self.off = 0  # column offset inside the (possibly shared) sc tile
        self.mate = None  # pair predecessor when this is a PAIR_SECOND chunk


def build_program():
    exp4 = _register_exp4()
    divf = _register_divf()
    nc = bacc.Bacc("TRN2", target_bir_lowering=False, debug=False)

    qt_d = nc.dram_tensor("QT", [DM, S], MM, kind="ExternalInput").ap()
    kt_d = nc.dram_tensor("KT", [DM, S], MM, kind="ExternalInput").ap()
    vt_d = nc.dram_tensor("VT", [DM, S], MM, kind="ExternalInput").ap()
    # host pre-transposed: [p, which(k/q), j, col]
    wkq_d = nc.dram_tensor("WKQ", [P, 2, DM // P, COLS], MM, kind="ExternalInput").ap()
    wv_d = nc.dram_tensor("WV", [P, DM // P, COLS], MM, kind="ExternalInput").ap()
    wo_d = nc.dram_tensor("WO", [P, COLS // P, DM], MM, kind="ExternalInput").ap()
    im_d = nc.dram_tensor("IM", [P, 2 * P], MM, kind="ExternalInput").ap()
    bkq_d = nc.dram_tensor("BKQ", [P, 2, COLS // P], FP32, kind="ExternalInput").ap()
    out_d = nc.dram_tensor("OUT", [S, DM], MM, kind="ExternalOutput").ap()

    NJ = DM // P  # 4 dm slabs
    NT = S // 512  # 4 seq tiles

    with tile.TileContext(nc) as tc, ExitStack() as ctx:
        const = ctx.enter_context(tc.tile_pool(name="const", bufs=1))
        persist = ctx.enter_context(tc.tile_pool(name="persist", bufs=1))

        # ---- constants (gpsimd ring), host-pre-transposed so every DMA
        # is [128 rows x >=1KB contiguous] ----
        wkq_sb = const.tile([P, 2, NJ, COLS], MM, tag="wkq")
        nc.gpsimd.dma_start(out=wkq_sb[:], in_=wkq_d[:, :, :, :])
        bkq_sb = const.tile([P, 2, COLS // P], FP32, tag="bkq")
        nc.gpsimd.dma_start(out=bkq_sb[:], in_=bkq_d[:, :, :])
        im_sb = const.tile([P, 2 * P], MM, tag="im")
        nc.gpsimd.dma_start(out=im_sb[:], in_=im_d[:, :])
        idn_sb = im_sb[:, 0:P]
        msk_sb = im_sb[:, P : 2 * P]
        wv_sb = const.tile([P, NJ, COLS], MM, tag="wv")  # after vt_h1 below
        wo_sb = const.tile([P, COLS // P, DM], MM, tag="wo")
        # per-partition ln(g) bias for the ACT exp
        lng_sb = const.tile([P, 1], FP32, tag="lng")
        nc.vector.memset(lng_sb[:], LN_G)

        # ---- persistent activations ----
        qt_st = [persist.tile([P, S], MM, tag=f"qst{i}", name=f"qst{i}") for i in range(2)]
        kt_st = [persist.tile([P, S], MM, tag=f"kst{i}", name=f"kst{i}") for i in range(2)]
        v_sb = persist.tile([P, NKB, HC, DK + 1], MM, tag="vaug")
        nc.vector.memset(v_sb[:, :, :, DK : DK + 1], 1.0)
        ctxt_sb = [persist.tile([P, S], MM, tag=f"ctxt{i}", name=f"ctxt{i}") for i in range(2)]
        # per-head q/k with the 64 head dims duplicated onto BOTH partition
        # halves. The duplication doubles the scores (folded into SCALE/8)
        # and, critically, keeps the score matmuls at full 128-row PE
        # activity: the HAM clock governor reads 64-row matmuls as a
        # half-idle array and clamps the PE to 1.2 GHz for ~23 windows at a
        # time (measured 16-on/23-off duty cycling, +30us on the kernel).
        qt_dup = [persist.tile([P, S], MM, tag=f"qtd{h}", name=f"qtd{h}") for h in range(HC)]
        kt_dup = [persist.tile([P, S], MM, tag=f"ktd{h}", name=f"ktd{h}") for h in range(HC)]

        # ================= Phase A: q/k projections =================
        xin = ctx.enter_context(tc.tile_pool(name="xin", bufs=4))
        kt_tiles = [xin.tile([P, S], MM, tag="x", bufs=4, name=f"kx{j}") for j in range(NJ)]
        qt_tiles = [xin.tile([P, S], MM, tag="xq", bufs=4, name=f"qx{j}") for j in range(NJ)]
        v_tiles = [xin.tile([P, S], MM, tag="xv", bufs=4, name=f"vt{j}") for j in range(NJ)]

        # sync ring: KT halves (low first -- smaller first transfers reach
        # the PE sooner on the shared HBM), then VT low half
        for hf in range(2):
            for j in range(NJ):
                nc.sync.dma_start(
                    out=kt_tiles[j][:, hf * 1024 : (hf + 1) * 1024],
                    in_=kt_d[j * P : (j + 1) * P, hf * 1024 : (hf + 1) * 1024],
                )
        for j in range(NJ):
            nc.sync.dma_start(
                out=v_tiles[j][:, 0:1024], in_=vt_d[j * P : (j + 1) * P, 0:1024]
            )
        # scalar ring: QT low half now; high half deferred below so KT/QT-lo
        # keep HBM priority
        for j in range(NJ):
            nc.scalar.dma_start(
                out=qt_tiles[j][:, 0:1024], in_=qt_d[j * P : (j + 1) * P, 0:1024]
            )
        # gpsimd ring: VT high half, then wv + wo (needed progressively later)
        for j in range(NJ):
            nc.gpsimd.dma_start(
                out=v_tiles[j][:, 1024:2048], in_=vt_d[j * P : (j + 1) * P, 1024:2048]
            )
        nc.gpsimd.dma_start(out=wv_sb[:], in_=wv_d[:, :, :])
        nc.gpsimd.dma_start(out=wo_sb[:], in_=wo_d[:, :, :])

        def dup(st, dst, h, q2):
            # duplicate the head's 64 dims onto both partition halves
            # for seq half q2 (sbuf->sbuf DMA; kt on gpsimd, qt on sync)
            eng = nc.gpsimd if st is kt_st else nc.sync
            qs = slice(q2 * (S // 2), (q2 + 1) * (S // 2))
            s_ap = st[h // 2][(h % 2) * DK : (h % 2) * DK + DK, qs]
            for half in range(2):
                eng.dma_start(
                    out=dst[h][half * DK : (half + 1) * DK, qs], in_=s_ap
                )

        def make_proj_group(pool, tag, bufs):
            def proj_group(which, t, cc, st, cb_engine):
                ps = pool.tile(
                    [P, 512], FP32, tag=tag, bufs=bufs, name=f"ps{which}{cc}_{t}"
                )
                for j in range(NJ):
                    nc.tensor.matmul(
                        ps[:],
                        wkq_sb[:, which, j, cc * P : (cc + 1) * P],
                        (kt_tiles if which == 0 else qt_tiles)[j][
                            :, t * 512 : (t + 1) * 512
                        ],
                        start=(j == 0),
                        stop=(j == NJ - 1),
                        skip_group_check=True,
                    )
                ts_ = slice(t * 512, (t + 1) * 512)
                if cb_engine == "act":
                    nc.scalar.activation(
                        st[cc][:, ts_],
                        ps[:],
                        AF.Identity,
                        bias=bkq_sb[:, which, cc : cc + 1],
                    )
                else:
                    nc.vector.tensor_scalar_add(
                        st[cc][:, ts_], ps[:], bkq_sb[:, which, cc : cc + 1]
                    )

            return proj_group

        with tc.tile_pool(name="pj_psum", bufs=8, space="PSUM") as pj_psum:
            proj_a = make_proj_group(pj_psum, "ps", 8)
            for t in range(NT):
                for cc in range(2):
                    proj_a(0, t, cc, kt_st, "act")
                if t == 0:
                    # release QT high-half DMAs after the t=0 K groups are
                    # emitted: scalar engine is in-order, so these issue
                    # after the first copybacks, giving the low halves HBM
                    # priority.
                    for jj in range(NJ):
                        nc.scalar.dma_start(
                            out=qt_tiles[jj][:, 1024:2048],
                            in_=qt_d[jj * P : (jj + 1) * P, 1024:2048],
                        )
                for cc in range(2):
                    proj_a(1, t, cc, qt_st, "act")
                if t == 1 or t == 3:
                    q2 = t // 2
                    for h in range(HC):
                        dup(kt_st, kt_dup, h, q2)
                        dup(qt_st, qt_dup, h, q2)

        # ================= Phase B: pipelined attention =================
        with tc.tile_pool(name="pb_psum", bufs=2, space="PSUM") as pb, tc.tile_pool(
            name="pt", bufs=4
        ) as pt_pool, tc.tile_pool(name="norm", bufs=4) as norm_pool, tc.tile_pool(
            name="osb", bufs=4
        ) as osb:
            po_tiles = {}

            def get_po(h, qq):
                key = (h, qq)
                if key not in po_tiles:
                    po_tiles[key] = pb.tile(
                        [DK + 1, 512], FP32, tag="po", bufs=4, name=f"po{h}_{qq}"
                    )
                return po_tiles[key]

            def vproj(tb):
                # rides the po-tag PSUM ring so the sc ring stays
                # double-buffered for the scores pipeline
                ps = pb.tile([P, HC, DK], FP32, tag="po", bufs=4, name=f"vps{tb}")
                for j in range(NJ):
                    nc.tensor.matmul(
                        ps[:, :, :],
                        v_tiles[j][:, tb * P : (tb + 1) * P],
                        wv_sb[:, j, :],
                        start=(j == 0),
                        stop=(j == NJ - 1),
                        skip_group_check=True,
                    )
                # v bias folded into bo on the host (softmax weights sum to 1)
                nc.vector.tensor_copy(v_sb[:, tb, :, 0:DK], ps[:, :, :])

            def oproj(tb):
                ps = pb.tile([P, DM], FP32, tag="po", bufs=4, name=f"ops{tb}")
                for cc in range(COLS // P):
                    nc.tensor.matmul(
                        ps[:],
                        ctxt_sb[cc][:, tb * P : (tb + 1) * P],
                        wo_sb[:, cc, :],
                        start=(cc == 0),
                        stop=(cc == COLS // P - 1),
                        skip_group_check=True,
                    )
                o = osb.tile([P, DM], MM, tag="o", name=f"o{tb}")
                if tb % 2:  # alternate the copy engines
                    nc.scalar.copy(o[:], ps[:])
                else:
                    nc.vector.tensor_copy(o[:], ps[:])
                # alternate output DMA rings too (drains in parallel)
                eng = nc.gpsimd if tb % 2 else nc.sync
                eng.dma_start(out=out_d[tb * P : (tb + 1) * P, :], in_=o[:])

            def normalize_cols(h, qq, off, width, done):
                """ctxT[head, cols] = po[0:64] / po[64] for `width` columns
                at quarter offset `off`: stage the sums row to partition 0,
                broadcast the RAW sums, then one fused approx-divide reading
                PSUM directly (recip seed + 1 Newton pass, ~0.17% max err)."""
                ti, po_ = h // 2, (h % 2) * DK
                po_q = get_po(h, qq)
                nm = f"{h}_{qq}_{off}"
                sums = norm_pool.tile([1, 512], FP32, tag="sums", name=f"s{nm}")
                nc.vector.tensor_copy(sums[:, :width], po_q[DK : DK + 1, off : off + width])
                bcast = norm_pool.tile([DK, 512], FP32, tag="bcast", name=f"b{nm}")
                nc.gpsimd.partition_broadcast(bcast[:, :width], sums[:, :width])
                g0 = qq * 512 + off
                if po_ == 0:
                    dst = ctxt_sb[ti][0:DK, g0 : g0 + width]
                else:
                    # custom-DVE ops cannot WRITE across partition bases
                    # (lanes are base-locked; measured garbage at base 64):
                    # stage at base 0 and move with a standard copy, which
                    # does handle the base change.
                    stage = norm_pool.tile([DK, 512], MM, tag="stg", name=f"g{nm}")
                    dst = stage[:, :width]
                nc.vector._custom_dve(
                    divf,
                    out=dst,
                    in0=po_q[0:DK, off : off + width],
                    in1=bcast[:, :width],
                    s0=DIV_C0,
                    s1=DIV_C1,
                )
                if po_ != 0:
                    nc.vector.tensor_copy(
                        ctxt_sb[ti][po_ : po_ + DK, g0 : g0 + width], dst
                    )
                if done:
                    del po_tiles[(h, qq)]

            def emit_scores(c):
                if c.mate is not None:
                    c.sc = c.mate.sc
                    c.off = c.mate.off + c.mate.w
                else:
                    c.sc = pb.tile(
                        [P, 1024], FP32, tag="sc", bufs=2, name=f"sc{c.h}_{c.kb}_{c.jt}"
                    )
                kt_h, qt_h = kt_dup[c.h], qt_dup[c.h]
                for lo, hi in _split512(c.w):
                    nc.tensor.matmul(
                        c.sc[:, c.off + lo : c.off + hi],
                        kt_h[:, c.q0 : c.q0 + P],
                        qt_h[:, c.tq0 + lo : c.tq0 + hi],
                        start=True,
                        stop=not (c.diag and lo == 0),
                        skip_group_check=True,
                    )
                if c.diag:  # add -30000 above the diagonal of the first block
                    nc.tensor.matmul(
                        c.sc[:, c.off : c.off + P],
                        idn_sb,
                        msk_sb,
                        start=False,
                        stop=True,
                        skip_group_check=True,
                    )

            def act_exp(c, lo, hi):
                nc.scalar.activation(
                    c.pt[:, lo:hi],
                    c.sc[:, lo:hi],
                    AF.Exp,
                    scale=4.0,
                    bias=lng_sb[:, 0:1],
                )

            def dve_exp(c, lo, hi):
                nc.vector._custom_dve(
                    exp4,
                    out=c.pt[:, lo:hi],
                    in0=c.sc[:, lo:hi],
                    s0=PA,
                    s1=PB,
                    imm2=PC,
                )

            def emit_exp(c):
                # for a pair, one exp covers both chunks' column ranges
                first = c.mate if c.mate is not None else c
                lo, hi = first.off, c.off + c.w
                c.pt = pt_pool.tile(
                    [P, 1024], MM, tag="pt", name=f"pt{c.h}_{c.kb}_{c.jt}"
                )
                if c.mate is not None:
                    c.mate.pt = c.pt
                if c.jt == 1 and c.kb in KB_DVE:
                    dve_exp(c, lo, hi)  # off-diagonal chunk: poly on the DVE
                elif (c.jt, c.kb) in SPLIT_DVE:
                    act_exp(c, lo, lo + P)  # masked diag block: hw exp -> 0
                    dve_exp(c, lo + P, hi)
                else:
                    act_exp(c, lo, hi)

            def pv_range(c, r0, r1):
                qc = (r0 // 512) * 512
                while qc < r1:
                    glo, ghi = max(r0, qc), min(r1, qc + 512)
                    qq = qc // 512
                    po_q = get_po(c.h, qq)
                    nc.tensor.matmul(
                        po_q[:, glo - qq * 512 : ghi - qq * 512],
                        v_sb[:, c.kb, c.h, :],
                        c.pt[:, c.off + glo - c.tq0 : c.off + ghi - c.tq0],
                        start=(c.kb == 0),
                        stop=(c.kb == LAST_KB[qq]),
                        skip_group_check=True,
                    )
                    qc += 512

            def emit_pv(c):
                if (c.jt, c.kb) in SPLIT_DVE:
                    # split at the exp engine boundary: the 128-col diag
                    # piece waits only on ACT, the tail only on the DVE
                    pv_range(c, c.tq0, c.tq0 + P)
                    pv_range(c, c.tq0 + P, c.tq0 + c.w)
                else:
                    pv_range(c, c.tq0, c.tq0 + c.w)

            # chunk stream: per head all jt=0 (low q) first, then jt=1
            chunks = []
            for h in range(HC):
                for kb in range(8):
                    chunks.append(_Chunk(h, kb, 0))
                for kb in range(NKB):
                    chunks.append(_Chunk(h, kb, 1))
            for i, c in enumerate(chunks):
                if (c.jt, c.kb) in PAIR_SECOND:
                    pj, pkb = PAIR_SECOND[(c.jt, c.kb)]
                    prev = chunks[i - 1]
                    assert (prev.jt, prev.kb) == (pj, pkb) and prev.h == c.h
                    c.mate = prev

            def pre_actions(i, c):
                # spread the late V-projection tiles across h0 (1 per 2 steps)
                if c.h == 0 and i % 2 == 0 and 4 + i // 2 < NKB:
                    vproj(4 + i // 2)

            def post_actions(c):
                h, kb, jt = c.h, c.kb, c.jt
                last = h == HC - 1
                if jt == 0 and kb == 3:
                    normalize_cols(h, 0, 0, 512, True)
                    if last:  # each oproj quad as soon as its quarter is done
                        for tb in range(0, 4):
                            oproj(tb)
                elif jt == 0 and kb == 7:
                    normalize_cols(h, 1, 0, 512, True)
                    if last:
                        for tb in range(4, 8):
                            oproj(tb)
                elif jt == 1 and kb == 11:
                    normalize_cols(h, 2, 0, 512, True)
                    if last:
                        for tb in range(8, 12):
                            oproj(tb)
                elif jt == 1 and kb == 15:
                    # drain: normalize the final quarter in 256-col pieces,
                    # each followed by its two output-projection tiles
                    normalize_cols(h, 3, 0, 256, False)
                    if last:
                        oproj(12)
                        oproj(13)
                    normalize_cols(h, 3, 256, 256, True)
                    if last:
                        oproj(14)
                        oproj(15)

            # V projection for key blocks 0-3 must precede the first PVs;
            # the rest stream in via pre_actions
            for tb in range(4):
                vproj(tb)

            # software-pipelined emission: scores of chunk i+1 land on the
            # PE queue before PV of chunk i, hiding the exp latency.
            # Pair-first chunks defer their exp+PV to the pair-second's
            # slot, where one merged exp covers both column ranges.
            is_first = {id(c.mate) for c in chunks if c.mate is not None}
            emit_scores(chunks[0])
            pre_actions(0, chunks[0])
            for i, c in enumerate(chunks):
                if i + 1 < len(chunks):
                    emit_scores(chunks[i + 1])
                    pre_actions(i + 1, chunks[i + 1])
                if id(c) in is_first:
                    continue
                emit_exp(c)
                if c.mate is not None:
                    emit_pv(c.mate)
                emit_pv(c)
                post_actions(c)

    nc.compile()
    return nc


def _get_nc():
    global _CACHED_NC
    if _CACHED_NC is None:
        _CACHED_NC = build_program()
    return _CACHED_NC


def make_in_maps(Q, K, V, Wq, bq, Wk, bk, Wv, bv, Wo, bo):
    f32 = lambda a: np.ascontiguousarray(a, dtype=np.float32)
    mm = lambda a: np.ascontiguousarray(np.asarray(a), dtype=MM_NP)
    qt = [mm(np.asarray(Q[b]).T) for b in range(B)]
    kt = [mm(np.asarray(K[b]).T) for b in range(B)]
    vt = [mm(np.asarray(V[b]).T) for b in range(B)]
    im = np.concatenate(
        [
            np.eye(P, dtype=MM_NP),
            np.tril(np.full((P, P), -30000.0, dtype=np.float32), -1).astype(MM_NP),
        ],
        axis=1,
    )
    qsc = SCALE / 8.0  # folded (incl. the 2x from dup) so PSUM scores are v
    maps = []
    for c in range(8):
        b, hg = c % B, c // B
        cs = slice(hg * COLS, (hg + 1) * COLS)
        wk_cs = np.asarray(Wk[:, cs], dtype=np.float32)
        wq_cs = np.asarray(Wq[:, cs], dtype=np.float32) * qsc
        wkq = (
            np.stack([wk_cs, wq_cs], axis=1)  # [512, 2, 256]
            .reshape(NJ_, P, 2, COLS)
            .transpose(1, 2, 0, 3)  # [128, 2, 4, 256]
        )
        wv_cs = np.asarray(Wv[:, cs], dtype=np.float32).reshape(NJ_, P, COLS).transpose(1, 0, 2)
        wo_cs = np.asarray(Wo[cs, :], dtype=np.float32).reshape(2, P, DM).transpose(1, 0, 2)
        bkq = np.stack(
            [
                f32(bk[cs]).reshape(2, P).T,
                (f32(bq[cs]) * qsc).reshape(2, P).T,
            ],
            axis=1,
        )  # [128, 2, 2]
        maps.append(
            {
                "QT": qt[b],
                "KT": kt[b],
                "VT": vt[b],
                "WKQ": mm(wkq),
                "WV": mm(wv_cs),
                "WO": mm(wo_cs),
                "IM": im,
                "BKQ": np.ascontiguousarray(bkq, dtype=np.float32),
            }
        )
    return maps


NJ_ = DM // P


def assemble(results, bv, Wo, bo):
    out = np.empty((B, S, DM), dtype=np.float32)
    for b in range(B):
        out[b] = results[b]["OUT"].astype(np.float32) + results[b + B][
            "OUT"
        ].astype(np.float32)
    # bv passes through attention unchanged (softmax rows sum to 1)
    bo_eff = np.asarray(bo, dtype=np.float32) + np.asarray(bv, np.float32) @ np.asarray(
        Wo, np.float32
    )
    out += bo_eff
    return out


def kernel(Q, K, V, Wq, bq, Wk, bk, Wv, bv, Wo, bo):
    nc = _get_nc()
    maps = make_in_maps(Q, K, V, Wq, bq, Wk, bk, Wv, bv, Wo, bo)
    res = run_bass_kernel_spmd(nc, maps, list(range(8)))
    return assemble(res.results, bv, Wo, bo)
